# revision 4
# baseline (speedup 1.0000x reference)
"""Trainium2 Bass kernel for nn_DMFLodel_8272107012191 (calibrated loss_fn).

Math (reference):
    occ      = max(class_occ, 1e-8)                      # [C]
    cal      = exp(logit - occ**-0.25)                   # [B, C]
    y_logit  = cal[b, y[b]]                              # [B]
    z_target = logit[b, y[b]]                            # [B]
    denom    = y_logit + sum_c(logit[b, :]) - z_target   # [B]
    loss     = mean_b(-log(divide_no_nan(y_logit, denom)))

Key observation: the full [B, C] calibrated matrix is never needed — only
the row sums of the raw logits plus two per-row gathers.  That makes this
a pure streaming-reduction problem: read logit once (206 MB total), plus
O(B) gathered elements.

Sharding: data-parallel over the batch axis.  8 cores x 128 rows each;
one batch row per SBUF partition.  Each core:
  1. loads its y indices and indirect-gathers logit[b, y[b]] and
     class_occ[y[b]] up front (tiny SWDGE transfers, fully overlapped),
  2. streams its [128, C] logit shard through SBUF in 4 MiB chunks on the
     HWDGE queue, reducing each chunk over the free axis on the vector
     engine,
  3. computes the per-row log-ratio in a handful of [128, 1] ops,
  4. sums the 128 per-row values on the PE (matmul with ones) and writes
     a single [1, 1] partial back to DRAM.
The host combines the 8 partials: loss = -(sum of partials) / B
(the all-reduce of the scalar loss collapses to this host-side sum).
"""

import numpy as np

B = 1024
C = 50257
N_CORES = 8
B_SH = B // N_CORES  # 128 rows per core == one SBUF partition each
TAU = 1.0
EPS = 1e-8

# streaming chunk width along the class axis (32 KiB/partition, 4 MiB/DMA)
CHUNK_W = 8192

_compiled = None  # compiled Bass module cache


def _build_nc():
    import concourse.bacc as bacc
    import concourse.bass as bass
    import concourse.tile as tile
    from concourse import mybir

    f32 = mybir.dt.float32
    i32 = mybir.dt.int32
    ACT = mybir.ActivationFunctionType

    nc = bacc.Bacc(
        "TRN2", target_bir_lowering=False, debug=False, num_devices=N_CORES
    )

    logit = nc.dram_tensor("logit", [B_SH, C], f32, kind="ExternalInput")
    yflat = nc.dram_tensor("yflat", [B_SH, 1], i32, kind="ExternalInput")
    y32 = nc.dram_tensor("y32", [B_SH, 1], i32, kind="ExternalInput")
    occ = nc.dram_tensor("class_occ", [C, 1], f32, kind="ExternalInput")
    loss_out = nc.dram_tensor("loss", [1, 1], f32, kind="ExternalOutput")

    chunks = []
    off = 0
    while off < C:
        w = min(CHUNK_W, C - off)
        chunks.append((off, w))
        off += w
    n_chunks = len(chunks)

    with tile.TileContext(nc) as tc:
        with (
            tc.tile_pool(name="stream", bufs=4) as stream_pool,
            tc.tile_pool(name="small", bufs=1) as small,
            tc.tile_pool(name="psum", bufs=1, space="PSUM") as psum_pool,
        ):
            # --- index loads + gathers first: tiny SWDGE transfers that must
            # not queue behind the 25.7 MB of streaming on the HWDGE path.
            yflat_t = small.tile([B_SH, 1], i32)
            nc.gpsimd.dma_start(out=yflat_t[:], in_=yflat[:, :])
            y_t = small.tile([B_SH, 1], i32)
            nc.gpsimd.dma_start(out=y_t[:], in_=y32[:, :])

            zt = small.tile([B_SH, 1], f32)
            nc.gpsimd.indirect_dma_start(
                out=zt[:],
                out_offset=None,
                in_=logit.ap().flatten().unsqueeze(1),
                in_offset=bass.IndirectOffsetOnAxis(ap=yflat_t[:, :1], axis=0),
            )
            occ_y = small.tile([B_SH, 1], f32)
            nc.gpsimd.indirect_dma_start(
                out=occ_y[:],
                out_offset=None,
                in_=occ.ap(),
                in_offset=bass.IndirectOffsetOnAxis(ap=y_t[:, :1], axis=0),
            )

            # pen = max(occ_y, EPS) ** -0.25  ==  1 / sqrt(sqrt(clamped))
            occ_c = small.tile([B_SH, 1], f32)
            nc.vector.tensor_scalar_max(out=occ_c[:], in0=occ_y[:], scalar1=EPS)
            s1 = small.tile([B_SH, 1], f32)
            nc.scalar.activation(out=s1[:], in_=occ_c[:], func=ACT.Sqrt)
            s2 = small.tile([B_SH, 1], f32)
            nc.scalar.activation(out=s2[:], in_=s1[:], func=ACT.Sqrt)
            pen = small.tile([B_SH, 1], f32)
            nc.vector.reciprocal(out=pen[:], in_=s2[:])

            # y_logit = exp(z_target - pen) == Exp(pen * -TAU + zt)
            ylog = small.tile([B_SH, 1], f32)
            nc.scalar.activation(
                out=ylog[:], in_=pen[:], func=ACT.Exp, bias=zt[:], scale=-TAU
            )

            # constants (scheduled early, no deps)
            zero = small.tile([B_SH, 1], f32)
            nc.vector.memset(zero[:], 0.0)
            ones = small.tile([B_SH, 1], f32)
            nc.vector.memset(ones[:], 1.0)

            # --- the memory-bound part: stream logit once, row-sum it.
            partials = small.tile([B_SH, n_chunks], f32)
            for i, (coff, w) in enumerate(chunks):
                t = stream_pool.tile([B_SH, CHUNK_W], f32, tag="stream")
                nc.sync.dma_start(out=t[:, :w], in_=logit[:, coff : coff + w])
                nc.vector.tensor_reduce(
                    out=partials[:, i : i + 1],
                    in_=t[:, :w],
                    axis=mybir.AxisListType.X,
                    op=mybir.AluOpType.add,
                )
            rowsum = small.tile([B_SH, 1], f32)
            nc.vector.tensor_reduce(
                out=rowsum[:],
                in_=partials[:],
                axis=mybir.AxisListType.X,
                op=mybir.AluOpType.add,
            )

            # denom = y_logit + (rowsum - z_target)
            den = small.tile([B_SH, 1], f32)
            nc.vector.tensor_tensor(
                out=den[:], in0=rowsum[:], in1=zt[:], op=mybir.AluOpType.subtract
            )
            nc.vector.tensor_tensor(
                out=den[:], in0=den[:], in1=ylog[:], op=mybir.AluOpType.add
            )

            # ratio = divide_no_nan(y_logit, denom)  (reciprocal + multiply)
            rden = small.tile([B_SH, 1], f32)
            nc.vector.reciprocal(out=rden[:], in_=den[:])
            ratio = small.tile([B_SH, 1], f32)
            nc.vector.tensor_tensor(
                out=ratio[:], in0=ylog[:], in1=rden[:], op=mybir.AluOpType.mult
            )
            mask = small.tile([B_SH, 1], mybir.dt.uint8)
            nc.vector.tensor_scalar(
                out=mask[:],
                in0=den[:],
                scalar1=0.0,
                scalar2=None,
                op0=mybir.AluOpType.is_equal,
            )
            nc.vector.copy_predicated(out=ratio[:], mask=mask[:], data=zero[:])

            # per-row log-ratio; host applies the leading minus sign
            lnr = small.tile([B_SH, 1], f32)
            nc.scalar.activation(out=lnr[:], in_=ratio[:], func=ACT.Ln)

            # partition-sum on the PE: [1,1] = lnr[128,1]^T @ ones[128,1]
            acc = psum_pool.tile([1, 1], f32)
            nc.tensor.matmul(
                out=acc[:], lhsT=lnr[:], rhs=ones[:], start=True, stop=True
            )
            out_sb = small.tile([1, 1], f32)
            nc.scalar.copy(out=out_sb[:], in_=acc[:])
            nc.sync.dma_start(out=loss_out[:, :], in_=out_sb[:])

    nc.compile()
    return nc


def _get_nc():
    global _compiled
    if _compiled is None:
        _compiled = _build_nc()
    return _compiled


def make_in_maps(logit, y, class_occ):
    """Build the 8 per-core input dicts from the full-size inputs."""
    logit = np.ascontiguousarray(np.asarray(logit, dtype=np.float32))
    y_i = np.asarray(y).astype(np.int32).reshape(B)
    occ_col = np.ascontiguousarray(
        np.asarray(class_occ, dtype=np.float32).reshape(C, 1)
    )
    row_base = np.arange(B_SH, dtype=np.int32) * C
    in_maps = []
    for c in range(N_CORES):
        sl = slice(c * B_SH, (c + 1) * B_SH)
        y_sh = y_i[sl]
        in_maps.append(
            {
                "logit": logit[sl],
                "yflat": (row_base + y_sh).reshape(B_SH, 1),
                "y32": y_sh.reshape(B_SH, 1).copy(),
                "class_occ": occ_col,
            }
        )
    return in_maps


def run_spmd(in_maps, **kwargs):
    from concourse.bass_utils import run_bass_kernel_spmd

    nc = _get_nc()
    return run_bass_kernel_spmd(nc, in_maps, core_ids=list(range(N_CORES)), **kwargs)


def kernel(logit, y, class_occ):
    res = run_spmd(make_in_maps(logit, y, class_occ))
    total = np.float32(0.0)
    for c in range(N_CORES):
        total = total + np.float32(res.results[c]["loss"].reshape(()))
    return np.float32(-total / np.float32(B))


# revision 5
# speedup vs baseline: 1.0098x; 1.0098x over previous
"""Trainium2 Bass kernel for nn_DMFLodel_8272107012191 (calibrated loss_fn).

Math (reference):
    occ      = max(class_occ, 1e-8)                      # [C]
    cal      = exp(logit - occ**-0.25)                   # [B, C]
    y_logit  = cal[b, y[b]]                              # [B]
    z_target = logit[b, y[b]]                            # [B]
    denom    = y_logit + sum_c(logit[b, :]) - z_target   # [B]
    loss     = mean_b(-log(divide_no_nan(y_logit, denom)))

Key observation: the full [B, C] calibrated matrix is never needed — only
the row sums of the raw logits plus two per-row gathers.  That makes this
a pure streaming-reduction problem: read logit once (206 MB total), plus
O(B) gathered elements.  Per row,
    -log(y_logit / denom) = pen - z_target + log(denom)
(with +inf when denom == 0, matching divide_no_nan + log), which avoids a
divide on the critical tail.

Sharding: data-parallel over the batch axis.  8 cores x 128 rows each;
one batch row per SBUF partition.  Engine plan per core:
  sync/HWDGE : 7 x 4 MiB streaming chunk loads of the logit shard
  vector     : ONLY the chunk row-sum reductions + the short final chain
               (the DVE runs in order; anything gather-dependent in front
               of the reductions would stall the whole stream)
  gpsimd     : y loads, the two indirect-DMA gathers, constants, clamp
  scalar/ACT : pen = exp(-0.25*ln(occ_y)), y_logit = exp(z - pen),
               final ln(denom), PSUM->SBUF copy
  tensor/PE  : 128-row partition sum via matmul with ones -> [1,1]
The host combines the 8 partials: loss = (sum of partials) / B
(the all-reduce of the scalar mean collapses to this host-side sum).
"""

import numpy as np

B = 1024
C = 50257
N_CORES = 8
B_SH = B // N_CORES  # 128 rows per core == one SBUF partition each
TAU = 1.0
EPS = 1e-8

# streaming chunk width along the class axis (32 KiB/partition, 4 MiB/DMA)
CHUNK_W = 8192

_compiled = None  # compiled Bass module cache


def _build_nc():
    import concourse.bacc as bacc
    import concourse.bass as bass
    import concourse.tile as tile
    from concourse import mybir

    f32 = mybir.dt.float32
    i32 = mybir.dt.int32
    ACT = mybir.ActivationFunctionType

    nc = bacc.Bacc(
        "TRN2", target_bir_lowering=False, debug=False, num_devices=N_CORES
    )

    logit = nc.dram_tensor("logit", [B_SH, C], f32, kind="ExternalInput")
    yflat = nc.dram_tensor("yflat", [B_SH, 1], i32, kind="ExternalInput")
    y32 = nc.dram_tensor("y32", [B_SH, 1], i32, kind="ExternalInput")
    occ = nc.dram_tensor("class_occ", [C, 1], f32, kind="ExternalInput")
    loss_out = nc.dram_tensor("loss", [1, 1], f32, kind="ExternalOutput")

    chunks = []
    off = 0
    while off < C:
        w = min(CHUNK_W, C - off)
        chunks.append((off, w))
        off += w
    n_chunks = len(chunks)

    with tile.TileContext(nc) as tc:
        with (
            tc.tile_pool(name="stream", bufs=4) as stream_pool,
            tc.tile_pool(name="small", bufs=1) as small,
            tc.tile_pool(name="psum", bufs=1, space="PSUM") as psum_pool,
        ):
            # --- the memory-bound stream: emitted FIRST so the DVE's
            # in-order reduction stream waits on nothing but its chunk DMA.
            partials = small.tile([B_SH, n_chunks], f32)
            stream_tiles = []
            for i, (coff, w) in enumerate(chunks):
                t = stream_pool.tile([B_SH, CHUNK_W], f32, tag="stream")
                nc.sync.dma_start(out=t[:, :w], in_=logit[:, coff : coff + w])
                nc.vector.tensor_reduce(
                    out=partials[:, i : i + 1],
                    in_=t[:, :w],
                    axis=mybir.AxisListType.X,
                    op=mybir.AluOpType.add,
                )

            # --- gathers + pre-chain, entirely off the DVE (gpsimd + ACT)
            yflat_t = small.tile([B_SH, 1], i32)
            nc.gpsimd.dma_start(out=yflat_t[:], in_=yflat[:, :])
            y_t = small.tile([B_SH, 1], i32)
            nc.gpsimd.dma_start(out=y_t[:], in_=y32[:, :])

            zt = small.tile([B_SH, 1], f32)
            nc.gpsimd.indirect_dma_start(
                out=zt[:],
                out_offset=None,
                in_=logit.ap().flatten().unsqueeze(1),
                in_offset=bass.IndirectOffsetOnAxis(ap=yflat_t[:, :1], axis=0),
            )
            occ_y = small.tile([B_SH, 1], f32)
            nc.gpsimd.indirect_dma_start(
                out=occ_y[:],
                out_offset=None,
                in_=occ.ap(),
                in_offset=bass.IndirectOffsetOnAxis(ap=y_t[:, :1], axis=0),
            )

            ones = small.tile([B_SH, 1], f32)
            nc.gpsimd.memset(ones[:], 1.0)
            inf_t = small.tile([B_SH, 1], f32)
            nc.gpsimd.memset(inf_t[:], float("inf"))

            occ_c = small.tile([B_SH, 1], f32)
            nc.gpsimd.tensor_scalar_max(out=occ_c[:], in0=occ_y[:], scalar1=EPS)

            # pen = occ_c ** -0.25 == exp(-0.25 * ln(occ_c))   (ACT only)
            lnocc = small.tile([B_SH, 1], f32)
            nc.scalar.activation(out=lnocc[:], in_=occ_c[:], func=ACT.Ln)
            pen = small.tile([B_SH, 1], f32)
            nc.scalar.activation(out=pen[:], in_=lnocc[:], func=ACT.Exp, scale=-0.25)
            # y_logit = exp(z_target - pen) == Exp(pen * -TAU + zt)
            ylog = small.tile([B_SH, 1], f32)
            nc.scalar.activation(
                out=ylog[:], in_=pen[:], func=ACT.Exp, bias=zt[:], scale=-TAU
            )
            # s = pen - z_target (ready early, off the DVE)
            s_t = small.tile([B_SH, 1], f32)
            nc.gpsimd.tensor_tensor(
                out=s_t[:], in0=pen[:], in1=zt[:], op=mybir.AluOpType.subtract
            )

            # --- final chain (DVE + one ACT ln + PE sum)
            rowsum = small.tile([B_SH, 1], f32)
            nc.vector.tensor_reduce(
                out=rowsum[:],
                in_=partials[:],
                axis=mybir.AxisListType.X,
                op=mybir.AluOpType.add,
            )
            # denom = (rowsum - z_target) + y_logit
            den = small.tile([B_SH, 1], f32)
            nc.vector.tensor_tensor(
                out=den[:], in0=rowsum[:], in1=zt[:], op=mybir.AluOpType.subtract
            )
            nc.vector.tensor_tensor(
                out=den[:], in0=den[:], in1=ylog[:], op=mybir.AluOpType.add
            )
            mask = small.tile([B_SH, 1], mybir.dt.uint8)
            nc.vector.tensor_scalar(
                out=mask[:],
                in0=den[:],
                scalar1=0.0,
                scalar2=None,
                op0=mybir.AluOpType.is_equal,
            )
            ln_den = small.tile([B_SH, 1], f32)
            nc.scalar.activation(out=ln_den[:], in_=den[:], func=ACT.Ln)
            # loss_row = pen - z_target + ln(denom);  +inf where denom == 0
            loss_row = small.tile([B_SH, 1], f32)
            nc.vector.tensor_tensor(
                out=loss_row[:], in0=s_t[:], in1=ln_den[:], op=mybir.AluOpType.add
            )
            nc.vector.copy_predicated(out=loss_row[:], mask=mask[:], data=inf_t[:])

            # partition-sum on the PE: [1,1] = loss_row[128,1]^T @ ones[128,1]
            acc = psum_pool.tile([1, 1], f32)
            nc.tensor.matmul(
                out=acc[:], lhsT=loss_row[:], rhs=ones[:], start=True, stop=True
            )
            out_sb = small.tile([1, 1], f32)
            nc.scalar.copy(out=out_sb[:], in_=acc[:])
            nc.sync.dma_start(out=loss_out[:, :], in_=out_sb[:])

    nc.compile()
    return nc


def _get_nc():
    global _compiled
    if _compiled is None:
        _compiled = _build_nc()
    return _compiled


def make_in_maps(logit, y, class_occ):
    """Build the 8 per-core input dicts from the full-size inputs."""
    logit = np.ascontiguousarray(np.asarray(logit, dtype=np.float32))
    y_i = np.asarray(y).astype(np.int32).reshape(B)
    occ_col = np.ascontiguousarray(
        np.asarray(class_occ, dtype=np.float32).reshape(C, 1)
    )
    row_base = np.arange(B_SH, dtype=np.int32) * C
    in_maps = []
    for c in range(N_CORES):
        sl = slice(c * B_SH, (c + 1) * B_SH)
        y_sh = y_i[sl]
        in_maps.append(
            {
                "logit": logit[sl],
                "yflat": (row_base + y_sh).reshape(B_SH, 1),
                "y32": y_sh.reshape(B_SH, 1).copy(),
                "class_occ": occ_col,
            }
        )
    return in_maps


def run_spmd(in_maps, **kwargs):
    from concourse.bass_utils import run_bass_kernel_spmd

    nc = _get_nc()
    return run_bass_kernel_spmd(nc, in_maps, core_ids=list(range(N_CORES)), **kwargs)


def kernel(logit, y, class_occ):
    res = run_spmd(make_in_maps(logit, y, class_occ))
    total = np.float32(0.0)
    for c in range(N_CORES):
        total = total + np.float32(res.results[c]["loss"].reshape(()))
    return np.float32(total / np.float32(B))


# revision 7
# speedup vs baseline: 1.0397x; 1.0297x over previous
"""Trainium2 Bass kernel for nn_DMFLodel_8272107012191 (calibrated loss_fn).

Math (reference):
    occ      = max(class_occ, 1e-8)                      # [C]
    cal      = exp(logit - occ**-0.25)                   # [B, C]
    y_logit  = cal[b, y[b]]                              # [B]
    z_target = logit[b, y[b]]                            # [B]
    denom    = y_logit + sum_c(logit[b, :]) - z_target   # [B]
    loss     = mean_b(-log(divide_no_nan(y_logit, denom)))

Key observation: the full [B, C] calibrated matrix is never needed — only
the row sums of the raw logits plus two per-row gathers.  That makes this
a pure streaming-reduction problem: read logit once (206 MB total), plus
O(B) gathered elements.  Per row,
    -log(y_logit / denom) = pen - z_target + log(denom)
(with +inf when denom == 0, matching divide_no_nan + log), which avoids a
divide on the critical tail.

Sharding: data-parallel over the batch axis.  8 cores x 128 rows each;
one batch row per SBUF partition.  Engine plan per core:
  sync/HWDGE : 7 x 4 MiB streaming chunk loads of the logit shard
  vector     : ONLY the chunk row-sum reductions + the short final chain
               (the DVE runs in order; anything gather-dependent in front
               of the reductions would stall the whole stream)
  gpsimd     : y loads, the two indirect-DMA gathers, constants, clamp
  scalar/ACT : pen = exp(-0.25*ln(occ_y)), y_logit = exp(z - pen),
               final ln(denom), PSUM->SBUF copy
  tensor/PE  : 128-row partition sum via matmul with ones -> [1,1]
The host combines the 8 partials: loss = (sum of partials) / B
(the all-reduce of the scalar mean collapses to this host-side sum).
"""

import numpy as np

B = 1024
C = 50257
N_CORES = 8
B_SH = B // N_CORES  # 128 rows per core == one SBUF partition each
TAU = 1.0
EPS = 1e-8

# streaming chunk widths along the class axis.  Front chunks are 4 MiB
# (32 KiB/partition); the end tapers so each tail chunk's DVE reduction
# (~1.06 ns/col) fits inside the DMA time of the data behind it
# (~1.48 ns/col at ~350 GB/s) — otherwise the last big reduce adds ~7 us
# of pure DVE catch-up after the final bytes land.
CHUNK_WIDTHS = [8192] * 5 + [1617, 4096, 2048, 1024, 512]
CHUNK_W = max(CHUNK_WIDTHS)
assert sum(CHUNK_WIDTHS) == C

_compiled = None  # compiled Bass module cache


def _build_nc():
    import concourse.bacc as bacc
    import concourse.bass as bass
    import concourse.tile as tile
    from concourse import mybir

    f32 = mybir.dt.float32
    i32 = mybir.dt.int32
    ACT = mybir.ActivationFunctionType

    nc = bacc.Bacc(
        "TRN2", target_bir_lowering=False, debug=False, num_devices=N_CORES
    )

    logit = nc.dram_tensor("logit", [B_SH, C], f32, kind="ExternalInput")
    yflat = nc.dram_tensor("yflat", [B_SH, 1], i32, kind="ExternalInput")
    y32 = nc.dram_tensor("y32", [B_SH, 1], i32, kind="ExternalInput")
    occ = nc.dram_tensor("class_occ", [C, 1], f32, kind="ExternalInput")
    loss_out = nc.dram_tensor("loss", [1, 1], f32, kind="ExternalOutput")

    chunks = []
    off = 0
    for w in CHUNK_WIDTHS:
        chunks.append((off, w))
        off += w
    n_chunks = len(chunks)

    with tile.TileContext(nc) as tc:
        with (
            tc.tile_pool(name="stream", bufs=4) as stream_pool,
            tc.tile_pool(name="small", bufs=1) as small,
            tc.tile_pool(name="psum", bufs=1, space="PSUM") as psum_pool,
        ):
            # --- the memory-bound stream: emitted FIRST so the DVE's
            # in-order reduction stream waits on nothing but its chunk DMA.
            partials = small.tile([B_SH, n_chunks], f32)
            stream_tiles = []
            for i, (coff, w) in enumerate(chunks):
                t = stream_pool.tile([B_SH, CHUNK_W], f32, tag="stream")
                nc.sync.dma_start(out=t[:, :w], in_=logit[:, coff : coff + w])
                nc.vector.tensor_reduce(
                    out=partials[:, i : i + 1],
                    in_=t[:, :w],
                    axis=mybir.AxisListType.X,
                    op=mybir.AluOpType.add,
                )

            # --- gathers + pre-chain, entirely off the DVE (gpsimd + ACT)
            yflat_t = small.tile([B_SH, 1], i32)
            nc.gpsimd.dma_start(out=yflat_t[:], in_=yflat[:, :])
            y_t = small.tile([B_SH, 1], i32)
            nc.gpsimd.dma_start(out=y_t[:], in_=y32[:, :])

            zt = small.tile([B_SH, 1], f32)
            nc.gpsimd.indirect_dma_start(
                out=zt[:],
                out_offset=None,
                in_=logit.ap().flatten().unsqueeze(1),
                in_offset=bass.IndirectOffsetOnAxis(ap=yflat_t[:, :1], axis=0),
            )
            occ_y = small.tile([B_SH, 1], f32)
            nc.gpsimd.indirect_dma_start(
                out=occ_y[:],
                out_offset=None,
                in_=occ.ap(),
                in_offset=bass.IndirectOffsetOnAxis(ap=y_t[:, :1], axis=0),
            )

            ones = small.tile([B_SH, 1], f32)
            nc.gpsimd.memset(ones[:], 1.0)
            inf_t = small.tile([B_SH, 1], f32)
            nc.gpsimd.memset(inf_t[:], float("inf"))

            occ_c = small.tile([B_SH, 1], f32)
            nc.gpsimd.tensor_scalar_max(out=occ_c[:], in0=occ_y[:], scalar1=EPS)

            # pen = occ_c ** -0.25 == exp(-0.25 * ln(occ_c))   (ACT only)
            lnocc = small.tile([B_SH, 1], f32)
            nc.scalar.activation(out=lnocc[:], in_=occ_c[:], func=ACT.Ln)
            pen = small.tile([B_SH, 1], f32)
            nc.scalar.activation(out=pen[:], in_=lnocc[:], func=ACT.Exp, scale=-0.25)
            # y_logit = exp(z_target - pen) == Exp(pen * -TAU + zt)
            ylog = small.tile([B_SH, 1], f32)
            nc.scalar.activation(
                out=ylog[:], in_=pen[:], func=ACT.Exp, bias=zt[:], scale=-TAU
            )
            # s = pen - z_target (ready early, off the DVE)
            s_t = small.tile([B_SH, 1], f32)
            nc.gpsimd.tensor_tensor(
                out=s_t[:], in0=pen[:], in1=zt[:], op=mybir.AluOpType.subtract
            )

            # --- final chain (DVE + one ACT ln + PE sum)
            rowsum = small.tile([B_SH, 1], f32)
            nc.vector.tensor_reduce(
                out=rowsum[:],
                in_=partials[:],
                axis=mybir.AxisListType.X,
                op=mybir.AluOpType.add,
            )
            # denom = (rowsum - z_target) + y_logit
            den = small.tile([B_SH, 1], f32)
            nc.vector.tensor_tensor(
                out=den[:], in0=rowsum[:], in1=zt[:], op=mybir.AluOpType.subtract
            )
            nc.vector.tensor_tensor(
                out=den[:], in0=den[:], in1=ylog[:], op=mybir.AluOpType.add
            )
            mask = small.tile([B_SH, 1], mybir.dt.uint8)
            nc.vector.tensor_scalar(
                out=mask[:],
                in0=den[:],
                scalar1=0.0,
                scalar2=None,
                op0=mybir.AluOpType.is_equal,
            )
            ln_den = small.tile([B_SH, 1], f32)
            nc.scalar.activation(out=ln_den[:], in_=den[:], func=ACT.Ln)
            # loss_row = pen - z_target + ln(denom);  +inf where denom == 0
            loss_row = small.tile([B_SH, 1], f32)
            nc.vector.tensor_tensor(
                out=loss_row[:], in0=s_t[:], in1=ln_den[:], op=mybir.AluOpType.add
            )
            nc.vector.copy_predicated(out=loss_row[:], mask=mask[:], data=inf_t[:])

            # partition-sum on the PE: [1,1] = loss_row[128,1]^T @ ones[128,1]
            acc = psum_pool.tile([1, 1], f32)
            nc.tensor.matmul(
                out=acc[:], lhsT=loss_row[:], rhs=ones[:], start=True, stop=True
            )
            out_sb = small.tile([1, 1], f32)
            nc.scalar.copy(out=out_sb[:], in_=acc[:])
            nc.sync.dma_start(out=loss_out[:, :], in_=out_sb[:])

    nc.compile()
    return nc


def _get_nc():
    global _compiled
    if _compiled is None:
        _compiled = _build_nc()
    return _compiled


def make_in_maps(logit, y, class_occ):
    """Build the 8 per-core input dicts from the full-size inputs."""
    logit = np.ascontiguousarray(np.asarray(logit, dtype=np.float32))
    y_i = np.asarray(y).astype(np.int32).reshape(B)
    occ_col = np.ascontiguousarray(
        np.asarray(class_occ, dtype=np.float32).reshape(C, 1)
    )
    row_base = np.arange(B_SH, dtype=np.int32) * C
    in_maps = []
    for c in range(N_CORES):
        sl = slice(c * B_SH, (c + 1) * B_SH)
        y_sh = y_i[sl]
        in_maps.append(
            {
                "logit": logit[sl],
                "yflat": (row_base + y_sh).reshape(B_SH, 1),
                "y32": y_sh.reshape(B_SH, 1).copy(),
                "class_occ": occ_col,
            }
        )
    return in_maps


def run_spmd(in_maps, **kwargs):
    from concourse.bass_utils import run_bass_kernel_spmd

    nc = _get_nc()
    return run_bass_kernel_spmd(nc, in_maps, core_ids=list(range(N_CORES)), **kwargs)


def kernel(logit, y, class_occ):
    res = run_spmd(make_in_maps(logit, y, class_occ))
    total = np.float32(0.0)
    for c in range(N_CORES):
        total = total + np.float32(res.results[c]["loss"].reshape(()))
    return np.float32(total / np.float32(B))
